# revision 13
# baseline (speedup 1.0000x reference)
"""Trainium2 Bass kernel for nn_MeshAutoencoder (gnn_message_passing).

Sharding: 8 cores = 2 batches x 4 face-quarters. Each core embeds its own
4096 faces; per-batch groups AllGather face embeds, do a count-sorted CSR
gather-reduce for the scatter_mean, AllGather vertex feats, then run the
conv decoder on a 4096+2*16 position window (halo recompute; only tiny SE
AllReduces cross cores).
"""

import sys

sys.path.insert(0, "/opt/trn_rl_repo")

import numpy as np
from math import pi

import concourse.bass as bass
import concourse.bacc as bacc
import concourse.tile as tile
from concourse import mybir
from concourse import bass_utils
from concourse.masks import make_identity

# ---------------- problem constants ----------------
B, NV, NF = 2, 8192, 16384
DIM = 192
DEC_DIMS = (128, 128, 192, 192, 256, 384)
INIT_K = 7
P = 128

F_OWN = 4096
V_OWN = 2048
HALO = 16
N_DEC = F_OWN + 2 * HALO          # 4128
NCOL = N_DEC + 2                  # +2 guard cols
NSLOT = 3 * N_DEC                 # 12384
NSLOT_PAD = ((NSLOT + 127) // 128) * 128   # 12416
QS = NSLOT_PAD // 128             # 97
GTW = 9 + NSLOT_PAD + 9           # G_T cols (guards both sides)

FEL_R = F_OWN + 1                 # 4097
FEG_R = 4 * FEL_R
VFL_R = V_OWN + 1                 # 2049
VFG_R = 4 * VFL_R

NT_SIZES = [512] * 8 + [32]
NT_STARTS = np.cumsum([0] + NT_SIZES)[:-1].tolist()

f32 = mybir.dt.float32
bf16 = mybir.dt.bfloat16
i32 = mybir.dt.int32
i16 = mybir.dt.int16

AF = mybir.ActivationFunctionType
OP = mybir.AluOpType

DT_X = bf16                       # decoder activation/weight dtype knob
DEBUG = False

_CACHE = {}


def _np_dt(dt):
    return np.dtype(mybir.dt.np(dt))


# ================= host-side index/weight prep =================

def _wrap16(idx_flat):
    n = idx_flat.shape[0]
    assert n % 16 == 0
    w = idx_flat.reshape(n // 16, 16).T
    return np.tile(w, (8, 1)).astype(np.int16)


def _host_prep(inputs):
    verts = np.asarray(inputs["vertices"], np.float32)
    faces = np.asarray(inputs["faces"], np.int32)
    coor_tab = np.asarray(inputs["coor_tab"], np.float32)
    angle_tab = np.asarray(inputs["angle_tab"], np.float32)
    area_tab = np.asarray(inputs["area_tab"], np.float32)
    normal_tab = np.asarray(inputs["normal_tab"], np.float32)
    proj_w = np.asarray(inputs["proj_w"], np.float32)
    proj_b = np.asarray(inputs["proj_b"], np.float32)
    init_w = np.asarray(inputs["init_w"], np.float32)
    init_b = np.asarray(inputs["init_b"], np.float32)
    blocks = inputs["blocks"]
    dtx = _np_dt(DT_X)

    temb = np.zeros((512, 64), np.float32)
    temb[0:128] = coor_tab
    temb[128:256] = normal_tab
    temb[256:384, :16] = angle_tab
    temb[384:512, :16] = area_tab

    # fields: 0-8 coor(j,x), 9-11 norm, 12-14 angle, 15 area
    W_pad = np.zeros((1024, DIM), np.float32)
    for f in range(9):
        W_pad[64 * f:64 * f + 64] = proj_w[64 * f:64 * f + 64]
    for j in range(3):
        W_pad[64 * (9 + j):64 * (9 + j) + 64] = proj_w[624 + 64 * j:624 + 64 * j + 64]
    for j in range(3):
        W_pad[64 * (12 + j):64 * (12 + j) + 16] = proj_w[576 + 16 * j:576 + 16 * j + 16]
    W_pad[64 * 15:64 * 15 + 16] = proj_w[816:832]

    verts_pad = np.zeros((B, NV, 64), np.float32)
    verts_pad[:, :, :3] = verts

    winit = np.zeros((128, INIT_K, 3, 2, 128), np.float32)
    for d in range(INIT_K):
        for j in range(3):
            for ct in range(2):
                cs = 128 if ct == 0 else 64
                winit[:cs, d, j, ct, :] = \
                    init_w[:, 192 * j + 128 * ct:192 * j + 128 * ct + cs, d].T

    def conv_w(w):
        cout, cin, k = w.shape
        n_ci = (cin + 127) // 128
        n_co = (cout + 127) // 128
        out = np.zeros((128, k, n_ci, n_co, 128), np.float32)
        for dk in range(k):
            for ci in range(n_ci):
                cs = min(128, cin - 128 * ci)
                for co in range(n_co):
                    os_ = min(128, cout - 128 * co)
                    out[:cs, dk, ci, co, :os_] = \
                        w[128 * co:128 * co + os_, 128 * ci:128 * ci + cs, dk].T
        return out

    def bias_w(b):
        cout = b.shape[0]
        n_co = (cout + 127) // 128
        out = np.zeros((128, n_co), np.float32)
        for co in range(n_co):
            os_ = min(128, cout - 128 * co)
            out[:os_, co] = b[128 * co:128 * co + os_]
        return out

    blk_prep = []
    din = DEC_DIMS[0]
    for bi, dout in enumerate(DEC_DIMS[1:]):
        p = blocks[bi]
        inner = max(dout // 4, 16)
        n_dt = (dout + 127) // 128
        d = {
            "w1": conv_w(np.asarray(p["w1"], np.float32)).astype(dtx),
            "b1": bias_w(np.asarray(p["b1"], np.float32)),
            "w2": conv_w(np.asarray(p["w2"], np.float32)).astype(dtx),
            "b2": bias_w(np.asarray(p["b2"], np.float32)),
            "din": din, "dout": dout, "inner": inner,
        }
        se1 = np.asarray(p["se_w1"], np.float32) / float(NF)
        se1_t = np.zeros((128, n_dt, inner), np.float32)
        for dt_ in range(n_dt):
            os_ = min(128, dout - 128 * dt_)
            se1_t[:os_, dt_, :] = se1[128 * dt_:128 * dt_ + os_, :]
        d["se1"] = se1_t
        se2 = np.asarray(p["se_w2"], np.float32)
        se2_t = np.zeros((128, n_dt, 128), np.float32)
        for dt_ in range(n_dt):
            os_ = min(128, dout - 128 * dt_)
            se2_t[:inner, dt_, :os_] = se2[:, 128 * dt_:128 * dt_ + os_]
        d["se2"] = se2_t
        sb1 = np.zeros((128, 1), np.float32)
        sb1[:inner, 0] = np.asarray(p["se_b1"], np.float32)
        d["sb1"] = sb1
        d["sb2"] = bias_w(np.asarray(p["se_b2"], np.float32))
        if "res_w" in p:
            d["rw"] = conv_w(np.asarray(p["res_w"], np.float32)).astype(dtx)
            rb = np.asarray(p["res_b"], np.float32)
            d["rb_row"] = np.zeros((1, 128 * n_dt), np.float32)
            d["rb_row"][0, :dout] = rb
            d["rbz"] = bool(np.all(rb == 0.0))
        blk_prep.append(d)
        din = dout

    # ---- per-batch graph structures ----
    per_batch = []
    for g in range(B):
        fg = faces[g]
        fv = fg.ravel()
        cnt = np.bincount(fv, minlength=NV)
        order = np.argsort(fv, kind="stable")
        starts = np.searchsorted(fv[order], np.arange(NV))
        ranks = np.empty(NV, np.int64)
        perms = []
        for q in range(4):
            c = cnt[V_OWN * q:V_OWN * (q + 1)]
            rho = np.argsort(-c, kind="stable")
            perms.append(rho)
            ranks[V_OWN * q + rho] = np.arange(V_OWN)
        vrow = (np.arange(NV) // V_OWN) * VFL_R + ranks
        per_batch.append((fg, cnt, order, starts, perms, vrow))

    Kmax = 0
    cnt_cores = []
    for g in range(B):
        cnt = per_batch[g][1]
        for q in range(4):
            c = cnt[V_OWN * q:V_OWN * (q + 1)]
            Kmax = max(Kmax, int(c.max()))
            cnt_cores.append(c)
    m_k = []
    for k in range(Kmax):
        mk = 1
        for c in cnt_cores:
            mk = max(mk, (int((c > k).sum()) + 127) // 128)
        m_k.append(mk)
    m_k[0] = V_OWN // 128

    cores = []
    for cid in range(8):
        g, q = cid // 4, cid % 4
        fg, cnt, order, starts, perms, vrow = per_batch[g]
        F0 = F_OWN * q

        own = fg[F0:F0 + F_OWN]
        v_pij = own.reshape(32, 128, 3)           # [i, p, j]
        fc_list = v_pij.transpose(0, 2, 1).reshape(-1)   # r = (3i+j)*128+p
        idx_fc = _wrap16(fc_list.astype(np.int16))

        rho = perms[q]
        myv = V_OWN * q + rho
        mycnt = cnt[myv]
        csr_blocks = []
        for k in range(Kmax):
            n = 128 * m_k[k]
            arr = np.full(n, F_OWN, np.int64)     # pad -> zero row 4096
            sel = np.nonzero(mycnt > k)[0]
            fa = order[starts[myv[sel]] + k] // 3
            arr[sel] = (fa // F_OWN) * FEL_R + (fa % F_OWN)
            csr_blocks.append(_wrap16(arr.astype(np.int16)))
        csr_idx = np.concatenate(csr_blocks, axis=1)

        cnt_perm = np.zeros((128, 16), np.float32)
        cnt_perm[np.arange(V_OWN) % 128, np.arange(V_OWN) // 128] = mycnt

        w_arr = np.arange(N_DEC)
        fw = F0 - HALO + w_arr
        valid = (fw >= 0) & (fw < NF)
        rows = vrow[fg[np.clip(fw, 0, NF - 1)]].copy()   # [N_DEC, 3]
        rows[~valid] = V_OWN
        slot_list = np.full(NSLOT_PAD, V_OWN, np.int64)
        slot_list[:NSLOT] = rows.reshape(-1)
        idx_slot = _wrap16(slot_list.astype(np.int16))

        em = np.zeros((1, 32), np.float32)
        wl = np.arange(HALO)
        em[0, 0:HALO] = ((F0 - HALO + wl) >= 0).astype(np.float32)
        wr = F_OWN + HALO + np.arange(HALO)
        em[0, HALO:2 * HALO] = ((F0 - HALO + wr) < NF).astype(np.float32)
        cores.append({
            "verts_pad": verts_pad[g],
            "idx_fc": idx_fc,
            "csr_idx": csr_idx,
            "cnt_perm": cnt_perm,
            "idx_slot": idx_slot,
            "edge_mask": em,
        })

    angthr = np.cos(np.pi * np.arange(1, 128, dtype=np.float64) / 128.0)
    shared = {
        "temb": temb, "W_pad": W_pad, "proj_b": proj_b.reshape(1, DIM),
        "winit": winit.astype(dtx), "init_b": bias_w(init_b),
        "angthr": angthr.astype(np.float32).reshape(1, 127),
    }
    for bi, d in enumerate(blk_prep):
        for k, v in d.items():
            if isinstance(v, np.ndarray):
                shared[f"b{bi}_{k}"] = v
    meta = {"Kmax": Kmax, "m_k": tuple(m_k), "blocks": blk_prep}
    return cores, shared, meta


# ================= device program =================

def _ci_tiles(c):
    out = []
    i = 0
    while c > 0:
        out.append((i, min(128, c)))
        c -= 128
        i += 1
    return out


def _build(meta):
    Kmax = meta["Kmax"]
    m_k = list(meta["m_k"])
    blocks = meta["blocks"]
    CSRW = sum(8 * m for m in m_k)
    dt_x = DT_X

    nc = bacc.Bacc("TRN2", target_bir_lowering=False, debug=False,
                   num_devices=8)

    def din(name, shape, dtype=f32):
        return nc.dram_tensor(name, shape, dtype, kind="ExternalInput")

    verts_pad = din("verts_pad", [NV, 64])
    idx_fc = din("idx_fc", [128, 768], i16)
    csr_idx = din("csr_idx", [128, CSRW], i16)
    cnt_perm = din("cnt_perm", [128, 16])
    idx_slot = din("idx_slot", [128, NSLOT_PAD // 16], i16)
    edge_mask = din("edge_mask", [1, 32])
    angthr = din("angthr", [1, 127])
    temb = din("temb", [512, 64])
    W_pad = din("W_pad", [1024, DIM])
    proj_b = din("proj_b", [1, DIM])
    winit = din("winit", [128, INIT_K, 3, 2, 128], dt_x)
    init_b = din("init_b", [128, 1])
    binp = {}
    for bi, d in enumerate(blocks):
        for k, v in d.items():
            if isinstance(v, np.ndarray):
                dt_in = dt_x if k in ("w1", "w2", "rw") else f32
                binp[(bi, k)] = din(f"b{bi}_{k}", list(v.shape), dt_in)
    out_dec = nc.dram_tensor("out_dec", [384, F_OWN], dt_x,
                             kind="ExternalOutput")
    dbg_fe = nc.dram_tensor("dbg_fe", [FEL_R, DIM], f32,
                            kind="ExternalOutput") if DEBUG else None
    dbg_vf = nc.dram_tensor("dbg_vf", [VFG_R, DIM], f32,
                            kind="ExternalOutput") if DEBUG else None
    dbg_idx = nc.dram_tensor("dbg_idx", [128, 512], i32,
                             kind="ExternalOutput") if DEBUG else None

    RG = [[0, 1, 2, 3], [4, 5, 6, 7]]

    with tile.TileContext(nc) as tc:
        with (
            tc.tile_pool(name="cst", bufs=1) as cpool,
            tc.tile_pool(name="dram", bufs=1, space="DRAM") as dram,
            tc.tile_pool(name="gtp", bufs=1) as gtp,
        ):
            ident = cpool.tile([128, 128], f32)
            make_identity(nc, ident[:])
            ones_col = cpool.tile([128, 128], f32)
            nc.vector.memset(ones_col[:], 1.0)
            zrow = cpool.tile([1, DIM], f32)
            nc.vector.memset(zrow[:], 0.0)
            eps24 = cpool.tile([128, 1], f32)
            nc.vector.memset(eps24[:], 1e-24)
            eps12 = cpool.tile([128, 1], f32)
            nc.vector.memset(eps12[:], 1e-12)

            gta = gtp.tile([128, GTW], dt_x)
            gtb = gtp.tile([128, GTW], dt_x)

            # =============== phases A + B ===============
            with (
                tc.tile_pool(name="wkA", bufs=1) as wk,
                tc.tile_pool(name="wkD", bufs=2) as wkd,
                tc.tile_pool(name="psT", bufs=3, space="PSUM") as psT,
                tc.tile_pool(name="psF", bufs=2, space="PSUM") as psF,
            ):
                # ---- face coord gather ----
                idxfc_sb = wk.tile([128, 768], i16, tag="idxfc")
                nc.sync.dma_start(idxfc_sb[:], idx_fc[:])
                fcraw = wk.tile([128, 96, 64], f32, tag="big")
                for ch in range(3):
                    nc.gpsimd.dma_gather(
                        out_ap=fcraw[:, 32 * ch:32 * (ch + 1), :],
                        in_ap=verts_pad[:],
                        idxs_ap=idxfc_sb[:, 256 * ch:256 * (ch + 1)],
                        num_idxs=4096, num_idxs_reg=4096, elem_size=64,
                        single_packet=False)
                FC = wk.tile([128, 32, 3, 3], f32, tag="FC")
                nc.vector.tensor_copy(FC[:], fcraw[:, :, 0:3].rearrange(
                    "p (i j) x -> p i j x", j=3))

                idx_all = wk.tile([128, 32, 16], i32, tag="idxall")

                def disc(dst, src_ap, base, pre_add=None):
                    # u = (src [+pre_add]) * 64 - 0.5  (exactly jax's op
                    # order: power-of-2 scales), clip [0,127], round-half-even
                    # to int, then add the field base as an exact int op.
                    tmp = wk.tile([128, 32, 9], f32, tag="disc")
                    t = tmp[:, :, 0:src_ap.shape[-1]]
                    if pre_add is not None:
                        nc.vector.tensor_scalar_add(t, src_ap, float(pre_add))
                        nc.vector.tensor_scalar(t, t, 64.0, -0.5,
                                                OP.mult, OP.add)
                    else:
                        nc.vector.tensor_scalar(t, src_ap, 16.0, -0.5,
                                                OP.mult, OP.add)
                    nc.vector.tensor_scalar(t, t, 0.0, 127.0, OP.max, OP.min)
                    nc.vector.tensor_copy(dst, t)
                    if base:
                        nc.vector.tensor_scalar_add(dst, dst, base)

                disc(idx_all[:, :, 0:9],
                     FC[:].rearrange("p i j x -> p i (j x)"), 0, pre_add=1.0)

                SQ = wk.tile([128, 32, 3, 3], f32, tag="SQ")
                nc.scalar.activation(SQ[:], FC[:], AF.Square)
                SS = wk.tile([128, 32, 3], f32, tag="SS")
                nc.vector.reduce_sum(SS[:], SQ[:], axis=mybir.AxisListType.X)
                PRD = wk.tile([128, 32, 3, 3], f32, tag="PRD")
                nc.vector.tensor_mul(PRD[:, :, 1:3, :], FC[:, :, 1:3, :],
                                     FC[:, :, 0:2, :])
                nc.vector.tensor_mul(PRD[:, :, 0:1, :], FC[:, :, 0:1, :],
                                     FC[:, :, 2:3, :])
                DOTS = wk.tile([128, 32, 3], f32, tag="DOTS")
                nc.vector.reduce_sum(DOTS[:], PRD[:], axis=mybir.AxisListType.X)
                DN = wk.tile([128, 32, 3], f32, tag="DN")
                nc.vector.tensor_mul(DN[:, :, 1:3], SS[:, :, 1:3], SS[:, :, 0:2])
                nc.vector.tensor_mul(DN[:, :, 0:1], SS[:, :, 0:1], SS[:, :, 2:3])
                RS = wk.tile([128, 32, 3], f32, tag="RS")
                nc.scalar.activation(RS[:], DN[:], AF.Sqrt, bias=eps24[:, 0:1])
                nc.vector.reciprocal(RS[:], RS[:])
                COS = wk.tile([128, 32, 3], f32, tag="COS")
                nc.vector.tensor_mul(COS[:], DOTS[:], RS[:])
                nc.vector.tensor_scalar(COS[:], COS[:], -(1.0 - 1e-5),
                                        1.0 - 1e-5, OP.max, OP.min)
                # angle bins exactly, in cos space:
                # bin = sum_m [cos <= cos(pi*m/128)], m = 1..127
                thr_sb = wk.tile([1, 127], f32, tag="thr1")
                nc.sync.dma_start(thr_sb[:], angthr[:])
                thr_ps = psF.tile([128, 127], f32, tag="feps")
                nc.tensor.matmul(thr_ps[:], ones_col[0:1, :], thr_sb[:],
                                 start=True, stop=True)
                thr_bc = wk.tile([128, 127], f32, tag="thrbc")
                nc.vector.tensor_copy(thr_bc[:], thr_ps[:])
                for ich in range(4):
                    cmp = wk.tile([128, 8, 3, 127], f32, tag="cmp")
                    nc.vector.tensor_tensor(
                        cmp[:],
                        COS[:, 8 * ich:8 * (ich + 1), :, None].to_broadcast(
                            [128, 8, 3, 127]),
                        thr_bc[:, None, None, :].to_broadcast(
                            [128, 8, 3, 127]),
                        OP.is_le)
                    red = wk.tile([128, 8, 3], f32, tag="red")
                    nc.vector.reduce_sum(red[:], cmp[:],
                                         axis=mybir.AxisListType.X)
                    nc.vector.tensor_scalar_add(red[:], red[:], 256.0)
                    nc.vector.tensor_copy(
                        idx_all[:, 8 * ich:8 * (ich + 1), 12:15], red[:])

                E = wk.tile([128, 32, 2, 3], f32, tag="E")
                nc.vector.tensor_sub(E[:, :, 0, :], FC[:, :, 0, :],
                                     FC[:, :, 2, :])
                nc.vector.tensor_sub(E[:, :, 1, :], FC[:, :, 1, :],
                                     FC[:, :, 0, :])
                SH = wk.tile([128, 32, 4, 3], f32, tag="SH")
                nc.vector.tensor_copy(SH[:, :, 0, 0:2], E[:, :, 0, 1:3])
                nc.vector.tensor_copy(SH[:, :, 0, 2:3], E[:, :, 0, 0:1])
                nc.vector.tensor_copy(SH[:, :, 1, 0:1], E[:, :, 1, 2:3])
                nc.vector.tensor_copy(SH[:, :, 1, 1:3], E[:, :, 1, 0:2])
                nc.vector.tensor_copy(SH[:, :, 2, 0:1], E[:, :, 0, 2:3])
                nc.vector.tensor_copy(SH[:, :, 2, 1:3], E[:, :, 0, 0:2])
                nc.vector.tensor_copy(SH[:, :, 3, 0:2], E[:, :, 1, 1:3])
                nc.vector.tensor_copy(SH[:, :, 3, 2:3], E[:, :, 1, 0:1])
                # cross = fma(A, B, -fl(C*D)) matching XLA-CPU's fused
                # mul-sub (degenerate faces: the fma residual becomes the
                # normal after normalization). Dekker TwoProduct for A*B.
                CR = wk.tile([128, 32, 3], f32, tag="CR")
                P2 = wk.tile([128, 32, 3], f32, tag="P2")
                Am = SH[:, :, 0, :]
                Bm = SH[:, :, 1, :]
                nc.vector.tensor_mul(P2[:], SH[:, :, 2, :], SH[:, :, 3, :])
                nc.vector.tensor_mul(CR[:], Am, Bm)          # p = fl(A*B)
                Ah = wk.tile([128, 32, 3], f32, tag="Ah")
                Bh = wk.tile([128, 32, 3], f32, tag="Bh")
                nc.vector.tensor_scalar(
                    Ah[:].bitcast(i32), Am.bitcast(i32),
                    -4096, None, OP.bitwise_and)             # 0xFFFFF000
                nc.vector.tensor_scalar(
                    Bh[:].bitcast(i32), Bm.bitcast(i32),
                    -4096, None, OP.bitwise_and)
                Al = wk.tile([128, 32, 3], f32, tag="Al")
                Bl = wk.tile([128, 32, 3], f32, tag="Bl")
                nc.vector.tensor_sub(Al[:], Am, Ah[:])
                nc.vector.tensor_sub(Bl[:], Bm, Bh[:])
                ERR = wk.tile([128, 32, 3], f32, tag="ERR")
                TM = wk.tile([128, 32, 3], f32, tag="TM")
                nc.vector.tensor_mul(ERR[:], Ah[:], Bh[:])   # exact
                nc.vector.tensor_sub(ERR[:], ERR[:], CR[:])  # AhBh - p (exact)
                nc.vector.tensor_mul(TM[:], Ah[:], Bl[:])
                nc.vector.tensor_add(ERR[:], ERR[:], TM[:])
                nc.vector.tensor_mul(TM[:], Al[:], Bh[:])
                nc.vector.tensor_add(ERR[:], ERR[:], TM[:])
                nc.vector.tensor_mul(TM[:], Al[:], Bl[:])
                nc.vector.tensor_add(ERR[:], ERR[:], TM[:])  # e = A*B - p
                nc.vector.tensor_sub(CR[:], CR[:], P2[:])    # d = p - P
                nc.vector.tensor_add(CR[:], CR[:], ERR[:])   # round(A*B - P)
                CSQ = wk.tile([128, 32, 3], f32, tag="CSQ")
                nc.scalar.activation(CSQ[:], CR[:], AF.Square)
                CN2 = wk.tile([128, 32, 1], f32, tag="CN2")
                nc.vector.reduce_sum(CN2[:], CSQ[:], axis=mybir.AxisListType.X)
                RSN = wk.tile([128, 32, 1], f32, tag="RSN")
                nc.scalar.activation(RSN[:], CN2[:], AF.Sqrt, bias=eps24[:, 0:1])
                nc.vector.reciprocal(RSN[:], RSN[:])
                NRM = wk.tile([128, 32, 3], f32, tag="NRM")
                nc.vector.tensor_mul(NRM[:], CR[:],
                                     RSN[:].to_broadcast([128, 32, 3]))
                disc(idx_all[:, :, 9:12], NRM[:], 128, pre_add=1.0)
                AR2 = wk.tile([128, 32, 1], f32, tag="AR2")
                nc.scalar.activation(AR2[:], CN2[:], AF.Sqrt)
                disc(idx_all[:, :, 15:16], AR2[:], 384)

                if DEBUG:
                    nc.sync.dma_start(
                        dbg_idx[:], idx_all[:].rearrange("p i f -> p (i f)"))
                # wrap idx_all -> [128, 4096] i16
                idx16 = wk.tile([128, 512], i16, tag="idx16")
                nc.vector.tensor_copy(idx16[:],
                                      idx_all[:].rearrange("p i f -> p (i f)"))
                iscr = dram.tile([65536], i16)
                nc.sync.dma_start(
                    iscr[:].rearrange("(qq p) -> p qq", p=128), idx16[:])
                idxw = wk.tile([128, 4096], i16, tag="idxw")
                for h in range(8):
                    nc.sync.dma_start(
                        idxw[16 * h:16 * h + 16, :],
                        iscr[:].rearrange("(c pp) -> pp c", pp=16))

                wpad_sb = wk.tile([128, 8, DIM], f32, tag="wpad")
                nc.sync.dma_start(
                    wpad_sb[:], W_pad[:].rearrange("(t p) d -> p t d", p=128))
                pb_sb = wk.tile([1, DIM], f32, tag="pbsb")
                nc.sync.dma_start(pb_sb[:], proj_b[:])
                bias_ps = psF.tile([128, DIM], f32, tag="feps")
                nc.tensor.matmul(bias_ps[:], ones_col[0:1, :], pb_sb[:],
                                 start=True, stop=True)
                bias_bc = wk.tile([128, DIM], f32, tag="biasbc")
                nc.vector.tensor_copy(bias_bc[:], bias_ps[:])

                fe_sb = wk.tile([128, 32, DIM], f32, tag="fesb")
                for cc in range(16):
                    fraw = wkd.tile([128, 32, 64], f32, tag="wkbuf")
                    nc.gpsimd.dma_gather(
                        out_ap=fraw[:], in_ap=temb[:],
                        idxs_ap=idxw[:, 256 * cc:256 * (cc + 1)],
                        num_idxs=4096, num_idxs_reg=4096, elem_size=64,
                        single_packet=False)
                    frv = fraw[:].rearrange("p (i2 t) x -> p i2 (t x)", i2=2)
                    for i2 in range(2):
                        b = 2 * cc + i2
                        featT = wkd.tile([128, 8, 128], f32, tag="featT")
                        for t in range(8):
                            tp = psT.tile([128, 128], f32, tag="trps")
                            nc.tensor.transpose(
                                out=tp[:],
                                in_=frv[:, i2, 128 * t:128 * (t + 1)],
                                identity=ident[:])
                            nc.vector.tensor_copy(featT[:, t, :], tp[:])
                        fe_ps = psF.tile([128, DIM], f32, tag="feps")
                        for t in range(8):
                            nc.tensor.matmul(fe_ps[:], featT[:, t, :],
                                             wpad_sb[:, t, :],
                                             start=(t == 0), stop=(t == 7))
                        nc.vector.tensor_add(fe_sb[:, b, :], fe_ps[:],
                                             bias_bc[:])

                # ---- phase B ----
                feL = dram.tile([FEL_R, DIM], f32)
                nc.sync.dma_start(
                    feL[0:F_OWN, :].rearrange("(i p) d -> p i d", p=128),
                    fe_sb[:])
                nc.sync.dma_start(feL[F_OWN:F_OWN + 1, :], zrow[:])
                feG = dram.tile([FEG_R, DIM], f32)
                nc.gpsimd.collective_compute(
                    "AllGather", OP.bypass, replica_groups=RG,
                    ins=[feL[:]], outs=[feG[:]])
                if DEBUG:
                    nc.sync.dma_start(dbg_fe[:], feL[:])

                csr_sb = wk.tile([128, CSRW], i16, tag="csrsb")
                nc.sync.dma_start(csr_sb[:], csr_idx[:])
                acc = wk.tile([128, 16, DIM], f32, tag="acc")
                off = 0
                for k in range(Kmax):
                    mk = m_k[k]
                    if k == 0:
                        nc.gpsimd.dma_gather(
                            out_ap=acc[:, 0:mk, :], in_ap=feG[:],
                            idxs_ap=csr_sb[:, off:off + 8 * mk],
                            num_idxs=128 * mk, num_idxs_reg=128 * mk,
                            elem_size=DIM, single_packet=False)
                    else:
                        gk = wkd.tile([128, max(m_k[1:]), DIM], f32, tag="wkbuf")
                        nc.gpsimd.dma_gather(
                            out_ap=gk[:, 0:mk, :], in_ap=feG[:],
                            idxs_ap=csr_sb[:, off:off + 8 * mk],
                            num_idxs=128 * mk, num_idxs_reg=128 * mk,
                            elem_size=DIM, single_packet=False)
                        nc.vector.tensor_add(acc[:, 0:mk, :], acc[:, 0:mk, :],
                                             gk[:, 0:mk, :])
                    off += 8 * mk

                cntp = wk.tile([128, 16], f32, tag="cntp")
                nc.sync.dma_start(cntp[:], cnt_perm[:])
                rcp = wk.tile([128, 16], f32, tag="rcp")
                nc.vector.tensor_scalar_max(rcp[:], cntp[:], 1e-5)
                nc.vector.reciprocal(rcp[:], rcp[:])
                nc.vector.tensor_mul(
                    acc[:], acc[:],
                    rcp[:, :, None].to_broadcast([128, 16, DIM]))

                vfL = dram.tile([VFL_R, DIM], f32)
                nc.sync.dma_start(
                    vfL[0:V_OWN, :].rearrange("(q p) d -> p q d", p=128),
                    acc[:])
                nc.sync.dma_start(vfL[V_OWN:V_OWN + 1, :], zrow[:])
                vfG = dram.tile([VFG_R, DIM], f32)
                nc.gpsimd.collective_compute(
                    "AllGather", OP.bypass, replica_groups=RG,
                    ins=[vfL[:]], outs=[vfG[:]])
                if DEBUG:
                    nc.sync.dma_start(dbg_vf[:], vfG[:])

                islot_sb = wk.tile([128, NSLOT_PAD // 16], i16, tag="islot")
                nc.sync.dma_start(islot_sb[:], idx_slot[:])
                nc.vector.memset(gta[:, 0:9], 0.0)
                nc.vector.memset(gtb[:, 0:9], 0.0)
                nc.vector.memset(gta[:, GTW - 9:GTW], 0.0)
                nc.vector.memset(gtb[:, GTW - 9:GTW], 0.0)
                qoff = 0
                for ch, qn in ((0, 49), (1, 48)):
                    gs = wk.tile([128, 49, DIM], f32, tag="big")
                    nc.gpsimd.dma_gather(
                        out_ap=gs[:, 0:qn, :], in_ap=vfG[:],
                        idxs_ap=islot_sb[:, 8 * qoff:8 * (qoff + qn)],
                        num_idxs=128 * qn, num_idxs_reg=128 * qn,
                        elem_size=DIM, single_packet=False)
                    for qq in range(qn):
                        col = 9 + 128 * (qoff + qq)
                        tp = psT.tile([128, 128], f32, tag="trps")
                        nc.tensor.transpose(out=tp[:], in_=gs[:, qq, 0:128],
                                            identity=ident[:])
                        nc.vector.tensor_copy(gta[:, col:col + 128], tp[:])
                        tp2 = psT.tile([64, 128], f32, tag="trps2")
                        nc.tensor.transpose(out=tp2[:], in_=gs[:, qq, 128:192],
                                            identity=ident[:])
                        nc.vector.tensor_copy(gtb[0:64, col:col + 128], tp2[:])
                    qoff += qn

            # =============== phase C: decoder ===============
            with (
                tc.tile_pool(name="xp", bufs=3) as xp,
                tc.tile_pool(name="wp", bufs=1) as wp,
                tc.tile_pool(name="ps", bufs=4, space="PSUM") as ps,
                tc.tile_pool(name="psS", bufs=2, space="PSUM") as psS,
                tc.tile_pool(name="psZ", bufs=1, space="PSUM") as psZ,
                tc.tile_pool(name="sml", bufs=4) as sml,
                tc.tile_pool(name="dram2", bufs=2, space="DRAM") as dram2,
            ):
                em_sb = sml.tile([1, 32], f32, tag="emsb")
                nc.sync.dma_start(em_sb[:], edge_mask[:])
                em_ps = psZ.tile([128, 32], f32, tag="emps")
                nc.tensor.matmul(em_ps[:], ones_col[0:1, :], em_sb[:],
                                 start=True, stop=True)
                embc = sml.tile([128, 32], f32, tag="embc")
                nc.vector.tensor_copy(embc[:], em_ps[:])

                def mask_edges(xt, n_co_m):
                    for com in range(n_co_m):
                        nc.vector.tensor_mul(
                            xt[:, com, 1:1 + HALO],
                            xt[:, com, 1:1 + HALO],
                            embc[:, 0:HALO])
                        nc.vector.tensor_mul(
                            xt[:, com, 1 + HALO + F_OWN:NCOL - 1],
                            xt[:, com, 1 + HALO + F_OWN:NCOL - 1],
                            embc[:, HALO:2 * HALO])

                def new_x():
                    t = xp.tile([128, 3, NCOL], dt_x, tag="x")
                    nc.vector.memset(t[:, :, 0:1], 0.0)
                    nc.vector.memset(t[:, :, NCOL - 1:NCOL], 0.0)
                    return t

                wi_sb = wp.tile([128, INIT_K, 3, 2, 128], dt_x, tag="wi")
                nc.sync.dma_start(wi_sb[:], winit[:])
                initb_sb = sml.tile([128, 1], f32, tag="initb")
                nc.sync.dma_start(initb_sb[:], init_b[:])
                x = new_x()
                for a, nt in zip(NT_STARTS, NT_SIZES):
                    cps = ps.tile([128, 512], f32, tag="conv")
                    first = True
                    for d in range(INIT_K):
                        for j in range(3):
                            col0 = 3 * a + 3 * d + j
                            for ct, cs in ((0, 128), (1, 64)):
                                g_t = gta if ct == 0 else gtb
                                last = (d == INIT_K - 1 and j == 2 and ct == 1)
                                nc.tensor.matmul(
                                    cps[:, 0:nt],
                                    wi_sb[0:cs, d, j, ct, :],
                                    g_t[0:cs, col0:col0 + 3 * nt:3],
                                    start=first, stop=last)
                                first = False
                    nc.scalar.activation(x[:, 0, 1 + a:1 + a + nt],
                                         cps[:, 0:nt], AF.Silu,
                                         bias=initb_sb[:, 0:1])
                mask_edges(x, 1)

                for bi, blk in enumerate(blocks):
                    dinn, dout, inner = blk["din"], blk["dout"], blk["inner"]
                    ci_t = _ci_tiles(dinn)
                    co_t = _ci_tiles(dout)
                    n_co = len(co_t)
                    w1 = wp.tile([128, 3, len(ci_t), n_co, 128], dt_x, tag="w1")
                    nc.sync.dma_start(w1[:], binp[(bi, "w1")][:])
                    b1 = sml.tile([128, 3], f32, tag="b1")
                    nc.sync.dma_start(b1[:, 0:n_co], binp[(bi, "b1")][:])
                    w2 = wp.tile([128, 3, n_co, n_co, 128], dt_x, tag="w2")
                    nc.sync.dma_start(w2[:], binp[(bi, "w2")][:])
                    b2 = sml.tile([128, 3], f32, tag="b2")
                    nc.sync.dma_start(b2[:, 0:n_co], binp[(bi, "b2")][:])
                    se1 = sml.tile([128, 3, 96], f32, tag="se1")
                    nc.sync.dma_start(se1[:, 0:n_co, 0:inner],
                                      binp[(bi, "se1")][:])
                    se2 = sml.tile([128, 3, 128], f32, tag="se2")
                    nc.sync.dma_start(se2[:, 0:n_co, :], binp[(bi, "se2")][:])
                    sb1 = sml.tile([128, 1], f32, tag="sb1")
                    nc.sync.dma_start(sb1[:], binp[(bi, "sb1")][:])
                    sb2 = sml.tile([128, 3], f32, tag="sb2")
                    nc.sync.dma_start(sb2[:, 0:n_co], binp[(bi, "sb2")][:])
                    has_res = (bi, "rw") in binp
                    if has_res:
                        rw = wp.tile([128, 1, len(ci_t), n_co, 128], dt_x,
                                     tag="rw")
                        nc.sync.dma_start(rw[:], binp[(bi, "rw")][:])

                    def conv_pn(x_in, w_sb, b_sb, cin_tiles, cout_tiles, C):
                        x_o = new_x()
                        for a, nt in zip(NT_STARTS, NT_SIZES):
                            cps_l = []
                            for co in range(len(cout_tiles)):
                                cps = ps.tile([128, 512], f32, tag="conv")
                                first = True
                                nmm = len(cin_tiles) * 3
                                m = 0
                                for ci, cs in cin_tiles:
                                    for dk in range(3):
                                        m += 1
                                        nc.tensor.matmul(
                                            cps[:, 0:nt],
                                            w_sb[0:cs, dk, ci, co, :],
                                            x_in[0:cs, ci, a + dk:a + dk + nt],
                                            start=first, stop=(m == nmm))
                                        first = False
                                cps_l.append(cps)
                            ssq = psS.tile([128, 512], f32, tag="ssq")
                            for co, (coi, cos) in enumerate(cout_tiles):
                                y2 = sml.tile([128, 512], f32, tag="y2")
                                nc.scalar.activation(
                                    y2[0:cos, 0:nt], cps_l[co][0:cos, 0:nt],
                                    AF.Square, bias=b_sb[0:cos, co:co + 1])
                                nc.tensor.matmul(
                                    ssq[:, 0:nt], ones_col[0:cos, :],
                                    y2[0:cos, 0:nt], start=(co == 0),
                                    stop=(co == len(cout_tiles) - 1))
                            fbc = sml.tile([128, 512], f32, tag="fbc")
                            nc.scalar.activation(fbc[:, 0:nt], ssq[:, 0:nt],
                                                 AF.Sqrt, bias=eps12[:, 0:1],
                                                 scale=1.0 / C)
                            nc.vector.reciprocal(fbc[:, 0:nt], fbc[:, 0:nt])
                            for co, (coi, cos) in enumerate(cout_tiles):
                                tmid = sml.tile([128, 512], f32, tag="tmid")
                                nc.vector.scalar_tensor_tensor(
                                    out=tmid[0:cos, 0:nt],
                                    in0=cps_l[co][0:cos, 0:nt],
                                    scalar=b_sb[0:cos, co:co + 1],
                                    in1=fbc[0:cos, 0:nt],
                                    op0=OP.add, op1=OP.mult)
                                nc.scalar.activation(
                                    x_o[0:cos, co, 1 + a:1 + a + nt],
                                    tmid[0:cos, 0:nt], AF.Silu)
                        mask_edges(x_o, len(cout_tiles))
                        return x_o

                    xa = conv_pn(x, w1, b1, ci_t, co_t, dout)
                    h = conv_pn(xa, w2, b2, co_t, co_t, dout)

                    av = sml.tile([128, 3], f32, tag="av")
                    for co, (coi, cos) in enumerate(co_t):
                        nc.vector.reduce_sum(
                            av[0:cos, co:co + 1],
                            h[0:cos, co, 1 + HALO:1 + HALO + F_OWN],
                            axis=mybir.AxisListType.X)
                    se_in = dram2.tile([128, n_co], f32)
                    se_out = dram2.tile([128, n_co], f32)
                    nc.sync.dma_start(se_in[:], av[:, 0:n_co])
                    nc.gpsimd.collective_compute(
                        "AllReduce", OP.add, replica_groups=RG,
                        ins=[se_in[:]], outs=[se_out[:]])
                    av2 = sml.tile([128, 3], f32, tag="av2")
                    nc.sync.dma_start(av2[:, 0:n_co], se_out[:])
                    z1p = psZ.tile([128, 2], f32, tag="z1")
                    for co, (coi, cos) in enumerate(co_t):
                        nc.tensor.matmul(z1p[0:inner, 0:1],
                                         se1[0:cos, co, 0:inner],
                                         av2[0:cos, co:co + 1],
                                         start=(co == 0), stop=(co == n_co - 1))
                    z1 = sml.tile([128, 1], f32, tag="z1s")
                    nc.scalar.activation(z1[0:inner, :], z1p[0:inner, 0:1],
                                         AF.Silu, bias=sb1[0:inner, :])
                    gate = sml.tile([128, 3], f32, tag="gate")
                    for co, (coi, cos) in enumerate(co_t):
                        gp = psZ.tile([128, 2], f32, tag="z1")
                        nc.tensor.matmul(gp[0:cos, 0:1],
                                         se2[0:inner, co, 0:cos],
                                         z1[0:inner, :], start=True, stop=True)
                        nc.scalar.activation(gate[0:cos, co:co + 1],
                                             gp[0:cos, 0:1], AF.Sigmoid,
                                             bias=sb2[0:cos, co:co + 1])

                    if has_res:
                        x_n = new_x()
                        for a, nt in zip(NT_STARTS, NT_SIZES):
                            for co, (coi, cos) in enumerate(co_t):
                                rps = ps.tile([128, 512], f32, tag="conv")
                                m = 0
                                for ci, cs in ci_t:
                                    m += 1
                                    nc.tensor.matmul(
                                        rps[:, 0:nt], rw[0:cs, 0, ci, co, :],
                                        x[0:cs, ci, 1 + a:1 + a + nt],
                                        start=(m == 1), stop=(m == len(ci_t)))
                                nc.vector.scalar_tensor_tensor(
                                    out=x_n[0:cos, co, 1 + a:1 + a + nt],
                                    in0=h[0:cos, co, 1 + a:1 + a + nt],
                                    scalar=gate[0:cos, co:co + 1],
                                    in1=rps[0:cos, 0:nt],
                                    op0=OP.mult, op1=OP.add)
                        x = x_n
                    else:
                        for a, nt in zip(NT_STARTS, NT_SIZES):
                            for co, (coi, cos) in enumerate(co_t):
                                nc.vector.scalar_tensor_tensor(
                                    out=x[0:cos, co, 1 + a:1 + a + nt],
                                    in0=h[0:cos, co, 1 + a:1 + a + nt],
                                    scalar=gate[0:cos, co:co + 1],
                                    in1=x[0:cos, co, 1 + a:1 + a + nt],
                                    op0=OP.mult, op1=OP.add)

                nc.sync.dma_start(
                    out_dec[:].rearrange("(t p) n -> p t n", p=128),
                    x[:, :, 1 + HALO:1 + HALO + F_OWN])

    nc.compile()
    return nc


# ================= runner =================

def kernel(**inputs) -> np.ndarray:
    cores, shared, meta = _host_prep(inputs)
    key = (meta["Kmax"], meta["m_k"])
    if key not in _CACHE:
        _CACHE[key] = _build(meta)
    nc = _CACHE[key]

    in_maps = []
    for cid in range(8):
        m = dict(shared)
        m.update(cores[cid])
        in_maps.append({k: np.ascontiguousarray(v) for k, v in m.items()})

    res = bass_utils.run_bass_kernel_spmd(nc, in_maps, core_ids=list(range(8)))
    outs = [np.asarray(res.results[c]["out_dec"], np.float32)
            for c in range(8)]
    return np.stack([
        np.concatenate(outs[0:4], axis=1),
        np.concatenate(outs[4:8], axis=1),
    ])


def get_nc(**inputs):
    """Build (cached) program + in_maps without running — for test harness."""
    cores, shared, meta = _host_prep(inputs)
    key = (meta["Kmax"], meta["m_k"])
    if key not in _CACHE:
        _CACHE[key] = _build(meta)
    in_maps = []
    for cid in range(8):
        m = dict(shared)
        m.update(cores[cid])
        in_maps.append({k: np.ascontiguousarray(v) for k, v in m.items()})
    return _CACHE[key], in_maps
